# revision 33
# baseline (speedup 1.0000x reference)
"""Multi-head attention (shared key head) on 8 TRN2 NeuronCores.

Sharding: core c handles batch b = c % 4 and head group g = c // 4
(heads 4g..4g+3).  Per-core weights are sliced on host; x is
pre-transposed on host (bf16 for the V path, fp8e4 in DoubleRow
subtile layout for the Q/K path) so the device never transposes
activations.

Device-side per core:
  Q/K projections run as fp8e4 DoubleRow matmuls (contraction 256 per
  instruction), halving projection PE time; V projection stays bf16
  (fp8 V fails the accuracy budget).  qt/ktz are stored fp8, so the
  scores matmuls are fp8 x fp8 (throughput identical to bf16 at
  128-contraction, but cheaper SBUF traffic).
  KT is zero-padded into two 128-partition variants (low/high half) so
  every scores matmul contracts over the full 128 partitions.
  scores^T[k, q] = KTz^T . QT  (k on partitions, q moving, 1024-chunks)
  causal: k-tiles beyond the chunk's causal extent skipped, diagonal
  tiles restrict the moving range, one triangular -2048 mask-add
  matmul on the 128-wide boundary block (finite so the Schraudolph
  int16 path needs no saturation).
  exp: per (head, q-chunk) chunks are statically assigned to either
  the scalar engine (exact table exp) or the vector engine (one
  tensor_scalar op computing the Schraudolph bit-trick: bf16 bits of
  exp(s) ~ int16(s * SCALE*log2e*128 + 127*128 + c*128)).  Softmax
  normalization cancels the Schraudolph scale/sawtooth within a chunk,
  keeping rel-err at the exact-exp level.
  out^T[o(+denom), q] accumulates Vaug^T . attn^T in PSUM; Vaug blocks
  are 66 wide ([V+bv | ones | pad]) so AV LDWEIGHTS cost is halved;
  the ones column yields the softmax denominator in row 64.
  Epilogue: reciprocal_approx_fast on the PSUM denom row, GPSIMD
  partition-broadcast, one DVE multiply; output stays [o, q] and the
  host transposes it during the unshard.
"""

import math
import numpy as np
import ml_dtypes

import concourse.bass as bass
import concourse.mybir as mybir
import concourse.tile as tile
from concourse import bacc
from concourse.bass_utils import run_bass_kernel_spmd

B, S, D = 4, 2048, 512
H, A, O = 8, 64, 64
NCORES = 8
HPC = 4                # heads per core
APC = HPC * A          # 256 projection cols per core
VBLK = 66              # per-head V block width (64 out + 1 ones + 1 pad)
SCALE = 1.0 / math.sqrt(S)

F32 = mybir.dt.float32
BF16 = mybir.dt.bfloat16
FP8 = mybir.dt.float8e4
I16 = mybir.dt.int16
AF = mybir.ActivationFunctionType
ALU = mybir.AluOpType
BF_NP = ml_dtypes.bfloat16
F8_NP = ml_dtypes.float8_e4m3

QC = 1024              # attention q-chunk width
N_QC = S // QC         # 2
N_DT = D // 128        # 4 contraction tiles
N_ST = S // 128        # 16 s-tiles / k-tiles of 128

MASKV = 2048.0         # finite causal mask magnitude (exp(-2048*SCALE)~2e-20)
# Schraudolph exp on DVE: bf16 bits of exp(s*SCALE) ~ s*SMUL + SADD
C_SHIFT = 0.0      # 0 keeps Schraudolph scale ~1.0 so ACT/DVE tiles mix
SMUL = SCALE * math.log2(math.e) * 128.0
SADD = 127.0 * 128.0 + C_SHIFT * 128.0

# (h, qc) chunks whose exp runs on the vector engine (Schraudolph);
# chunk-granular so ACT and DVE don't contend on PSUM/SBUF ports
DVE_EXP = {(1, 0), (3, 0), (1, 1)}


def build():
    nc = bacc.Bacc("TRN2", target_bir_lowering=False, debug=False,
                   num_devices=NCORES)

    x8_d = nc.dram_tensor("x8", [128, N_DT, S], FP8, kind="ExternalInput").ap()
    xT_d = nc.dram_tensor("xT", [D, S], BF16, kind="ExternalInput").ap()
    wq8_d = nc.dram_tensor("wq8", [128, N_DT, APC], FP8,
                           kind="ExternalInput").ap()
    wk8_d = nc.dram_tensor("wk8", [128, N_DT, A], FP8,
                           kind="ExternalInput").ap()
    wv_d = nc.dram_tensor("wv", [D, APC], BF16, kind="ExternalInput").ap()
    bq_d = nc.dram_tensor("bq", [2, 128, 1], F32, kind="ExternalInput").ap()
    bvm_d = nc.dram_tensor("bvm", [128, APC], BF16, kind="ExternalInput").ap()
    vini_d = nc.dram_tensor("vini", [128, HPC * VBLK], BF16,
                            kind="ExternalInput").ap()
    out_d = nc.dram_tensor("out", [HPC, N_QC, O, QC], F32,
                           kind="ExternalOutput").ap()

    ngI_d = nc.inline_tensor((np.eye(128) * -MASKV).astype(BF_NP), "ngI").ap()
    mlt_np = (np.arange(128)[None, :] < np.arange(128)[:, None])
    mlt_d = nc.inline_tensor(mlt_np.astype(BF_NP), "mlt").ap()

    with tile.TileContext(nc) as tc:
        with tc.tile_pool(name="const", bufs=1) as cpool, \
             tc.tile_pool(name="persist", bufs=1) as ppool, \
             tc.tile_pool(name="attn", bufs=52) as apool, \
             tc.tile_pool(name="fin", bufs=3) as fpool, \
             tc.tile_pool(name="ps_sc", bufs=2, space="PSUM") as ps_sc, \
             tc.tile_pool(name="ps_av", bufs=2, space="PSUM") as ps_av:

            SY, SC, GP, VE = nc.sync, nc.scalar, nc.gpsimd, nc.vector

            # ---- SBUF tiles ----
            ngI = cpool.tile([128, 128], BF16, tag="ngI", name="ngI")
            mlt = cpool.tile([128, 128], BF16, tag="mlt", name="mlt")
            bvm = cpool.tile([128, APC], BF16, tag="bvm", name="bvm")
            vini = cpool.tile([128, HPC * VBLK], BF16, tag="vini", name="vini")

            x8 = cpool.tile([128, N_DT, S], FP8, tag="x8", name="x8")
            wq8 = cpool.tile([128, N_DT, APC], FP8, tag="wq8", name="wq8")
            wk8 = cpool.tile([128, N_DT, A], FP8, tag="wk8", name="wk8")
            bq_sb = [cpool.tile([128, 1], F32, tag=f"bq{at}", name=f"bq{at}")
                     for at in range(2)]
            wv_sb = [cpool.tile([128, APC], BF16, tag=f"wv{dt}",
                                name=f"wv{dt}") for dt in range(N_DT)]
            xth = [[ppool.tile([128, QC], BF16, tag=f"xt{dt}_{sp}",
                               name=f"xt{dt}_{sp}") for sp in range(2)]
                   for dt in range(N_DT)]

            qt8 = [[ppool.tile([128, QC], FP8, tag=f"qt{at}_{sp}",
                               name=f"qt{at}_{sp}") for sp in range(2)]
                   for at in range(2)]
            ktz = [[ppool.tile([128, QC], FP8, tag=f"ktz{i}_{sp}",
                               name=f"ktz{i}_{sp}") for sp in range(2)]
                   for i in range(2)]
            vt = [ppool.tile([128, HPC * VBLK], BF16, tag=f"v{st}",
                             name=f"v{st}") for st in range(N_ST)]

            # ---- DMA issue order / queues ----
            # scalar queue: only the two tiny consts the warmup needs
            SC.dma_start(out=ngI[:, :], in_=ngI_d[:, :])
            SC.dma_start(out=mlt[:, :], in_=mlt_d[:, :])
            # sync queue: K/Q-path critical inputs, then bf16 x tiles
            SY.dma_start(out=wk8[:, :, :], in_=wk8_d[:, :, :])
            SY.dma_start(out=x8[:, :, 0:QC], in_=x8_d[:, :, 0:QC])
            for dt in range(N_DT):
                SY.dma_start(out=xth[dt][0][:, :],
                             in_=xT_d[dt * 128:(dt + 1) * 128, 0:QC])
            for dt in range(N_DT):
                SY.dma_start(out=xth[dt][1][:, :],
                             in_=xT_d[dt * 128:(dt + 1) * 128, QC:S])
            # gpsimd queue: sp0 ktz zero halves (needed by the first scores
            # matmuls), Q weights, bias, second x8 half, then V-path
            nc.gpsimd.memset(ktz[0][0][64:128, :], 0.0)
            nc.gpsimd.memset(ktz[1][0][0:64, :], 0.0)
            GP.dma_start(out=vini[:, :], in_=vini_d[:, :])
            GP.dma_start(out=wq8[:, :, :], in_=wq8_d[:, :, :])
            GP.dma_start(out=x8[:, :, QC:S], in_=x8_d[:, :, QC:S])
            for at in range(2):
                GP.dma_start(out=bq_sb[at][:, :], in_=bq_d[at])
            nc.gpsimd.memset(ktz[0][1][64:128, :], 0.0)
            nc.gpsimd.memset(ktz[1][1][0:64, :], 0.0)
            GP.dma_start(out=bvm[:, :], in_=bvm_d[:, :])
            for dt in range(N_DT):
                GP.dma_start(out=wv_sb[dt][:, :],
                             in_=wv_d[dt * 128:(dt + 1) * 128, :])

            # vt init: ones/pad columns via fast DVE copies of the pattern
            for st in range(N_ST):
                nc.vector.tensor_copy(vt[st][:, :], vini[:, :])

            # PE warm-up: dense back-to-back matmuls on real const data
            # (all-zero operands don't register as HAM activity) bridge the
            # activity window so the clock un-throttles early
            wu = ps_sc.tile([128, 512], F32, tag="sc", name="wu")
            for i in range(20):
                nc.tensor.matmul(out=wu[:, 0:128], lhsT=ngI[:, :],
                                 rhs=mlt[:, :], start=True, stop=True)

            # dummy exp pulls the ACT exp-table load off the critical path
            tw = fpool.tile([128, 1], F32, tag="tw", name="tw")
            nc.scalar.activation(out=tw[:, :], in_=ngI[:, 0:1],
                                 func=AF.Exp, scale=0.0)

            # ---- projections ----
            def qt_proj(at, sp):
                """QT[a, s] for head pair at via fp8 DoubleRow; bias+cast
                evac on the scalar engine."""
                ps = ps_av.tile([128, QC], F32, tag="av", name="qps")
                for hh in range(2):
                    hs = slice(sp * QC + hh * 512, sp * QC + (hh + 1) * 512)
                    for j in range(2):
                        nc.tensor.matmul(
                            out=ps[:, hh * 512:(hh + 1) * 512],
                            lhsT=wq8[:, 2 * j:2 * j + 2,
                                     at * 128:(at + 1) * 128],
                            rhs=x8[:, 2 * j:2 * j + 2, hs],
                            perf_mode=mybir.MatmulPerfMode.DoubleRow,
                            start=(j == 0), stop=(j == 1))
                nc.scalar.activation(out=qt8[at][sp][:, :], in_=ps[:, :],
                                     func=AF.Identity, bias=bq_sb[at][:, :],
                                     scale=1.0)

            def kt_proj(sp):
                """KT[a, s]; evac into both zero-padded variants (scalar
                engine for the unshifted copy, DVE for the shifted one)."""
                ps = ps_av.tile([64, QC], F32, tag="av", name="kps")
                for hh in range(2):
                    hs = slice(sp * QC + hh * 512, sp * QC + (hh + 1) * 512)
                    for j in range(2):
                        nc.tensor.matmul(
                            out=ps[:, hh * 512:(hh + 1) * 512],
                            lhsT=wk8[:, 2 * j:2 * j + 2, :],
                            rhs=x8[:, 2 * j:2 * j + 2, hs],
                            perf_mode=mybir.MatmulPerfMode.DoubleRow,
                            start=(j == 0), stop=(j == 1))
                nc.scalar.activation(out=ktz[0][sp][0:64, :], in_=ps[:, :],
                                     func=AF.Copy)
                nc.vector.tensor_copy(ktz[1][sp][64:128, :], ps[:, :])

            def v_proj(st):
                """V s-tile -> vt[st] blocks [V+bv | 1 | 0] (bf16)."""
                ps = ps_av.tile([128, APC], F32, tag="av", name="vps")
                sp, so = st // 8, (st % 8) * 128
                for dt in range(N_DT):
                    nc.tensor.matmul(
                        out=ps[:, :],
                        lhsT=xth[dt][sp][:, so:so + 128],
                        rhs=wv_sb[dt][:, :],
                        start=(dt == 0), stop=(dt == N_DT - 1))
                v3 = vt[st][:, :].rearrange("p (h c) -> p h c", h=HPC)
                p3 = ps[:, :].rearrange("p (h c) -> p h c", h=HPC)
                b3 = bvm[:, :].rearrange("p (h c) -> p h c", h=HPC)
                nc.vector.tensor_add(out=v3[:, :, 0:O], in0=p3[:, :, :],
                                     in1=b3[:, :, :])

            # ---- attention ----
            def sc_exp(h, qc, kjs=None, atns=None):
                """scores + exp for one (head, q-chunk); returns atn tiles.
                kjs restricts to a kj subrange so other work can be woven
                between exp batches of a big chunk."""
                at = h // 2
                on_dve = (h, qc) in DVE_EXP
                nkj = (QC // 128) * (qc + 1)
                atns = [] if atns is None else atns
                for kj in (range(nkj) if kjs is None else kjs):
                    m = kj - (QC // 128) * qc
                    vs = 128 * m if m > 0 else 0     # valid q start
                    sc_ps = ps_sc.tile([128, QC], F32, tag="sc", name="sc")
                    for hf in range(QC // 512):
                        lo = max(vs, hf * 512)
                        hi = (hf + 1) * 512
                        if lo >= hi:
                            continue
                        nc.tensor.matmul(
                            out=sc_ps[:, lo:hi],
                            lhsT=ktz[h % 2][kj // 8][:, (kj % 8) * 128:
                                                     (kj % 8 + 1) * 128],
                            rhs=qt8[at][qc][:, lo:hi],
                            start=True, stop=True)
                    if m >= 0:
                        nc.tensor.matmul(out=sc_ps[:, vs:vs + 128],
                                         lhsT=ngI[:, :], rhs=mlt[:, :],
                                         start=False, stop=True,
                                         skip_group_check=True)
                    atn = apool.tile([128, QC], BF16, tag="atn", name="atn")
                    if on_dve:
                        nc.vector.tensor_scalar(
                            out=atn[:, vs:QC].bitcast(I16),
                            in0=sc_ps[:, vs:QC],
                            scalar1=SMUL, scalar2=SADD,
                            op0=ALU.mult, op1=ALU.add)
                    else:
                        nc.scalar.activation(out=atn[:, vs:QC],
                                             in_=sc_ps[:, vs:QC],
                                             func=AF.Exp, scale=SCALE)
                    atns.append(atn)
                return atns

            def av_mms(h, qc, atns):
                """V-weighted accumulation matmuls for one chunk"""
                av = ps_av.tile([128, QC], F32, tag="av", name="av")
                nkj = (QC // 128) * (qc + 1)
                for kj in range(nkj):
                    m = kj - (QC // 128) * qc
                    vs = 128 * m if m > 0 else 0
                    for hf in range(QC // 512):
                        lo = max(vs, hf * 512)
                        hi = (hf + 1) * 512
                        if lo >= hi:
                            continue
                        last_kj = nkj - 1 if hf == 1 else \
                            (QC // 128) * qc + 3
                        nc.tensor.matmul(
                            out=av[0:VBLK, lo:hi],
                            lhsT=vt[kj][:, h * VBLK:(h + 1) * VBLK],
                            rhs=atns[kj][:, lo:hi],
                            start=(kj == 0), stop=(kj == last_kj))
                return av

            def av_ep(h, qc, av, oq, halves=1):
                """softmax-denominator epilogue; halves=2 splits the chain
                into q-halves so the tail pipelines across engines"""
                w = QC // halves
                for j in range(halves):
                    qs = slice(j * w, (j + 1) * w)
                    dr = fpool.tile([1, w], F32, tag="dr", name="dr")
                    nc.vector.tensor_copy(dr[:, :], av[O:O + 1, qs])
                    r = fpool.tile([1, w], F32, tag="r", name="r")
                    nc.vector.reciprocal_approx_fast(out=r[:, :],
                                                     in_=dr[:, :])
                    rb = fpool.tile([O, w], F32, tag="rb", name="rb")
                    nc.gpsimd.partition_broadcast(rb[:, :], r[:, :],
                                                  channels=O)
                    ov = fpool.tile([O, w], F32, tag="ov", name="ov")
                    nc.vector.tensor_mul(ov[:, :], av[0:O, qs], rb[:, :])
                    [SY, GP][oq].dma_start(
                        out=out_d[h, qc, :, qs], in_=ov[:, :])

            def av_part(h, qc, atns, oq, halves=1):
                av_ep(h, qc, av_mms(h, qc, atns), oq, halves)

            # ---- schedule ----
            # minimal projections -> early scores; DVE exp chunks emitted in
            # k-tile batches with projection/evacuation work between them;
            # the two scalar-engine qc=1 chunks run back-to-back so the
            # scalar engine never starves; AV work fills the PE in between.
            kt_proj(0)
            qt_proj(0, 0)
            a00 = sc_exp(0, 0)
            kt_proj(1)
            a10 = sc_exp(1, 0, kjs=range(0, 4))
            qt_proj(1, 0)
            a10 = sc_exp(1, 0, kjs=range(4, 8), atns=a10)
            a20 = sc_exp(2, 0)
            qt_proj(0, 1)
            a30 = sc_exp(3, 0, kjs=range(0, 4))
            qt_proj(1, 1)
            v_proj(0)
            v_proj(1)
            a30 = sc_exp(3, 0, kjs=range(4, 8), atns=a30)
            v_proj(2)
            v_proj(3)
            a01 = sc_exp(0, 1)
            for st in range(4, 8):
                v_proj(st)
            av00 = av_mms(0, 0, a00)
            av_ep(0, 0, av00, 0)
            av10 = av_mms(1, 0, a10)
            av_ep(1, 0, av10, 1)
            a21 = sc_exp(2, 1)
            av20 = av_mms(2, 0, a20)
            av_ep(2, 0, av20, 0)
            av30 = av_mms(3, 0, a30)
            av_ep(3, 0, av30, 1)
            for st in range(8, 12):
                v_proj(st)
            a11 = sc_exp(1, 1, kjs=range(0, 6))
            v_proj(12)
            v_proj(13)
            a11 = sc_exp(1, 1, kjs=range(6, 11), atns=a11)
            v_proj(14)
            v_proj(15)
            a11 = sc_exp(1, 1, kjs=range(11, 16), atns=a11)
            av01 = av_mms(0, 1, a01)
            av_ep(0, 1, av01, 0)
            a31 = sc_exp(3, 1)
            av11 = av_mms(1, 1, a11)
            av_ep(1, 1, av11, 1)
            av21 = av_mms(2, 1, a21)
            av_ep(2, 1, av21, 0, halves=2)
            av31 = av_mms(3, 1, a31)
            av_ep(3, 1, av31, 1, halves=2)

    nc.compile()
    return nc


_NC = None
LAST_RESULTS = None


def make_in_maps(x, Wq, bq, Wk, Wv, bv):
    vini_blk = np.zeros((HPC, VBLK), dtype=np.float32)
    vini_blk[:, O] = 1.0
    vini = np.ascontiguousarray(np.broadcast_to(
        vini_blk.reshape(1, HPC * VBLK), (128, HPC * VBLK))).astype(BF_NP)

    def dsub(a):     # [512, C] -> [128, 4, C] fp8 (partition, d-subtile, col)
        C = a.shape[1]
        return np.ascontiguousarray(
            a.reshape(N_DT, 128, C).transpose(1, 0, 2)).astype(F8_NP)

    in_maps = []
    for c in range(NCORES):
        b, g = c % 4, c // 4
        cols = slice(g * APC, (g + 1) * APC)
        xt = np.ascontiguousarray(x[b].T)
        bvc = np.asarray(bv[cols], dtype=np.float32)
        in_maps.append({
            "x8": dsub(xt),
            "xT": xt.astype(BF_NP),
            "wq8": dsub(Wq[:, cols]),
            "wk8": dsub(Wk),
            "wv": np.ascontiguousarray(Wv[:, cols]).astype(BF_NP),
            "bq": np.ascontiguousarray(bq[cols].reshape(2, 128, 1)),
            "bvm": np.ascontiguousarray(np.broadcast_to(
                bvc.reshape(1, APC), (128, APC))).astype(BF_NP),
            "vini": vini,
        })
    return in_maps


def gather_out(results):
    out = np.empty((B, S, H * O), dtype=np.float32)
    for c in range(NCORES):
        b, g = c % 4, c // 4
        oc = results[c]["out"]          # [HPC, N_QC, O, QC]
        for h in range(HPC):
            col = g * APC + h * O
            for qc in range(N_QC):
                out[b, qc * QC:(qc + 1) * QC, col:col + O] = oc[h, qc].T
    return out


def kernel(**inputs):
    global _NC, LAST_RESULTS
    x = np.asarray(inputs["x"], dtype=np.float32)
    Wq = np.asarray(inputs["Wq"], dtype=np.float32)
    bq = np.asarray(inputs["bq"], dtype=np.float32)
    Wk = np.asarray(inputs["Wk"], dtype=np.float32)
    Wv = np.asarray(inputs["Wv"], dtype=np.float32)
    bv = np.asarray(inputs["bv"], dtype=np.float32)

    if _NC is None:
        _NC = build()

    in_maps = make_in_maps(x, Wq, bq, Wk, Wv, bv)
    res = run_bass_kernel_spmd(_NC, in_maps, core_ids=list(range(NCORES)))
    LAST_RESULTS = res
    return gather_out(res.results)
